# revision 26
# baseline (speedup 1.0000x reference)
"""CapsuleLayer (dynamic routing) Trainium2 kernel, v2.

Math (see reference): u_hat[b,j,n,o] = sum_i x[b,n,i] W[j,n,i,o]; 3 routing
iterations of softmax-over-j (j=2 -> sigmoid of logit diff) + squash.

Design: shard the n axis (91392) over 8 cores. Per core (n=11424, padded to
11776 = 92*128):
  - s-type sums  t[b,(j,o)] = sum_{n,i} y[b,n,i] W[j,n,i,o]
      ONE full-array matmul per 128-n chunk: lhsT = y_c [128n, 128(i,b)],
      rhs = Ws_c [128n, 256(i,jo)], PSUM-accumulated outer product
      [128(i,b), 256(i',jo)]; the wanted i==i' diagonal blocks are summed
      with a 3-step DVE tree at the end (log-free vs 8x more matmuls).
  - logit pass   z[b,n,i] = sum_{j,o} +/-v[b,j,o] W[j,n,i,o]
      lhsT = S4 = I_4 (x) Vt^T (fp8), rhs = W4 (fp8) [128(il,j,o), n],
      two col-tiled matmuls (H halves) per 512-n tile.
    d[b,n] = sum_i x*z: xz elementwise (DVE, x8 fp8) + delta-matmul vs a
      [128,16] one-hot (PE), 4 x [128n,16b] into one [128,64] PSUM tile.
    w = sigmoid(d): ONE ACT op per tile [128,(k4,b)=64]; i-replication is
      done by a stride-0 broadcast read in the y = w*x DVE multiply.
  - Cross-core reduction of the [16,32] partials via AllGather (cheaper
    floor than AllReduce) + 3 DVE tree adds.
A pre-sync AllGather absorbs cross-core launch skew under the DMA phase.
Bulk loads ride the sync+gpsimd DMA queues; the scalar queue is kept free
for latency-critical small transfers (collective staging). Final partials
are combined on the host (tiny [16,32] math), as in the original design.
"""
import sys

sys.path.insert(0, "/opt/trn_rl_repo")

import numpy as np
import ml_dtypes

BF16 = ml_dtypes.bfloat16
FP8 = ml_dtypes.float8_e4m3
N_CORES = 8
B = 16
NIN = 91392
DI = 8
DO = 16
NC_N = NIN // N_CORES  # 11424
NCP = 11776  # 92 chunks of 128; 23 z-tiles of 512
CHUNKS = 92
ZT = 23
# XW DMA groups: (first_chunk, n_chunks)
GROUPS = [(g * 8, 8) for g in range(11)] + [(88, 4)]
EPS = 1e-7

_CACHE = {}


def host_prep(x, W, n_cores=N_CORES):
    """Split x [B,N,8] / W [2,N,8,16] into per-core packed layouts."""
    n_per = x.shape[1] // n_cores
    oneD = np.zeros((128, 16), dtype=BF16)
    for i in range(DI):
        for b in range(B):
            oneD[i * 16 + b, b] = 1.0
    in_maps = []
    for c in range(n_cores):
        xc = np.zeros((B, NCP, DI), dtype=np.float32)
        Wc = np.zeros((2, NCP, DI, DO), dtype=np.float32)
        xc[:, :n_per] = x[:, c * n_per : (c + 1) * n_per]
        Wc[:, :n_per] = W[:, c * n_per : (c + 1) * n_per]
        # xs[n128, (chunk, i, b)] = x[b, n, i]
        xs = (
            xc.reshape(B, CHUNKS, 128, DI)  # b c n i
            .transpose(2, 1, 3, 0)  # n c i b
            .reshape(128, CHUNKS, 128)
        )
        # Ws[n128, (chunk, i, j, o)] = W[j, n, i, o]
        Ws = (
            Wc.reshape(2, CHUNKS, 128, DI, DO)  # j c n i o
            .transpose(2, 1, 3, 0, 4)  # n c i j o
            .reshape(128, CHUNKS, 256)
        )
        # XW: per group [xs_g(nch*128) | ws_g(nch*256)]
        parts = []
        for g0, nch in GROUPS:
            parts.append(xs[:, g0 : g0 + nch].reshape(128, nch * 128))
            parts.append(Ws[:, g0 : g0 + nch].reshape(128, nch * 256))
        XW = np.ascontiguousarray(np.concatenate(parts, axis=1)).astype(BF16)
        # W4[(il, j, o), (t, H, ns)] = W[j, t*512+ns, H*4+il, o]  (fp8)
        W4 = np.ascontiguousarray(
            Wc.reshape(2, ZT, 512, 2, 4, DO)  # j t ns H il o
            .transpose(4, 0, 5, 1, 3, 2)  # il j o t H ns
            .reshape(128, ZT * 1024)
        ).astype(FP8)
        # x8[(i, b), n] = x[b, n, i]  (fp8)
        x8 = np.ascontiguousarray(
            xc.transpose(2, 0, 1).reshape(128, NCP)
        ).astype(FP8)
        in_maps.append({"XW": XW, "W4": W4, "x8": x8, "oneD": oneD})
    return in_maps


def build_kernel(num_devices=N_CORES):
    from contextlib import ExitStack

    import concourse.bacc as bacc
    import concourse.tile as tile
    from concourse import mybir

    DT = mybir.dt.bfloat16
    F8 = mybir.dt.float8e4
    F32 = mybir.dt.float32
    AF = mybir.ActivationFunctionType

    xw_cols = sum(nch * 384 for _, nch in GROUPS)  # 35328

    nc = bacc.Bacc(
        "TRN2", target_bir_lowering=False, debug=False, num_devices=num_devices
    )
    xw_in = nc.declare_dram_parameter("XW", [128, xw_cols], DT, isOutput=False)
    w4_in = nc.declare_dram_parameter("W4", [128, ZT * 1024], F8, isOutput=False)
    x8_in = nc.declare_dram_parameter("x8", [128, NCP], F8, isOutput=False)
    oned_in = nc.declare_dram_parameter("oneD", [128, 16], DT, isOutput=False)
    t2_out = nc.declare_dram_parameter("t2", [32, 64], F32, isOutput=True)
    s0g_out = nc.declare_dram_parameter("s0g", [16, 32], F32, isOutput=True)

    ag_bufs = []
    for k in range(3):
        ag_bufs.append(
            (
                nc.dram_tensor(f"ag_in{k}", [32, 64], F32),
                nc.dram_tensor(f"ag_out{k}", [256, 64], F32, addr_space="Shared"),
            )
        )
    pre_in = nc.dram_tensor("pre_in", [1, 16], F32)
    pre_out = nc.dram_tensor("pre_out", [8, 16], F32, addr_space="Shared")

    with tile.TileContext(nc) as tc, ExitStack() as ctx:
        park = ctx.enter_context(tc.tile_pool(name="park", bufs=1))
        ps_s = ctx.enter_context(tc.tile_pool(name="ps_s", bufs=2, space="PSUM"))
        ps_z = ctx.enter_context(tc.tile_pool(name="ps_z", bufs=3, space="PSUM"))
        ps_d = ctx.enter_context(tc.tile_pool(name="ps_d", bufs=2, space="PSUM"))
        work = ctx.enter_context(tc.tile_pool(name="work", bufs=4))
        xzpool = ctx.enter_context(tc.tile_pool(name="xzpool", bufs=ZT))
        ypool = ctx.enter_context(tc.tile_pool(name="ypool", bufs=3))
        wpool = ctx.enter_context(tc.tile_pool(name="wpool", bufs=ZT))
        small = ctx.enter_context(tc.tile_pool(name="small", bufs=2))

        # ---- pre-sync: first instructions. Kicks off the ncfw first-
        # collective setup (variable 40-100us) so it runs under the DMA/
        # stage-A phase instead of delaying AG0. The gathered values are
        # never read (tiny zero staged only to satisfy nonfinite checks).
        zt_sb = work.tile([1, 16], F32, tag="zt_sb")
        nc.vector.memset(zt_sb[:], 0.0)
        nc.scalar.dma_start(pre_in[:], zt_sb[:])
        nc.gpsimd.collective_compute(
            "AllGather",
            mybir.AluOpType.bypass,
            replica_groups=[list(range(num_devices))],
            ins=[pre_in[:]],
            outs=[pre_out[:]],
        )

        # ---- resident input tiles ----
        # XW rides all three DMA queues (scalar's share lands well before
        # the first collective staging needs it); the W4/x8 tail stays off
        # scalar so post-stage-A small transfers aren't queued behind bulk.
        xw_engs = [nc.sync, nc.scalar, nc.gpsimd]
        tail_engs = [nc.sync, nc.gpsimd]
        _rr = [0]

        def load_xw(dst_ap, src_ap):
            xw_engs[_rr[0] % 3].dma_start(dst_ap, src_ap)
            _rr[0] += 1

        _rr2 = [0]

        def load(dst_ap, src_ap):
            tail_engs[_rr2[0] % 2].dma_start(dst_ap, src_ap)
            _rr2[0] += 1

        xw_t = []
        off = 0
        for gi, (g0, nch) in enumerate(GROUPS):
            t = park.tile([128, nch * 384], DT, tag=f"xw{gi}")
            load_xw(t[:], xw_in[:, off : off + nch * 384])
            xw_t.append(t)
            off += nch * 384
        oneD = park.tile([128, 16], DT, tag="oneD")
        nc.scalar.dma_start(oneD[:], oned_in[:])
        # W4 + x8 interleaved in pass-1 consumption order. Queues chosen so
        # each queue's tail starts only after its XW share (strict XW
        # priority), and scalar's share lands early (free for AG0 staging).
        w4sb = park.tile([128, ZT * 1024], F8, tag="w4sb")
        x8sb = park.tile([128, NCP], F8, tag="x8sb")
        tail = [
            (w4sb[:, 0:4096], w4_in[:, 0:4096], nc.scalar),
            (w4sb[:, 4096:8192], w4_in[:, 4096:8192], nc.sync),
            (x8sb[:, 0:4096], x8_in[:, 0:4096], nc.gpsimd),
            (w4sb[:, 8192:12288], w4_in[:, 8192:12288], nc.scalar),
            (w4sb[:, 12288:16384], w4_in[:, 12288:16384], nc.sync),
            (x8sb[:, 4096:8192], x8_in[:, 4096:8192], nc.gpsimd),
            (w4sb[:, 16384:20480], w4_in[:, 16384:20480], nc.sync),
            (w4sb[:, 20480:23552], w4_in[:, 20480:23552], nc.gpsimd),
            (x8sb[:, 8192:11776], x8_in[:, 8192:11776], nc.sync),
        ]
        # Guard: tail loads must not steal HBM bandwidth from the XW stream
        # stage A chases. A 1-elem copy into EACH tail region reads the last
        # XW tile, so every tail DMA (WAW overlap) waits for the XW stream.
        for dst, src, eng in tail:
            nc.vector.tensor_copy(dst[0:1, 0:1], xw_t[-1][0:1, 0:1])
        for dst, src, eng in tail:
            eng.dma_start(dst, src)

        s4_tiles = {}
        for it in (1, 2):
            s4_tile = park.tile([128, 128], F8, tag=f"s4_{it}")
            nc.vector.memset(s4_tile[:], 0.0)
            s4_tiles[it] = s4_tile

        chunk_map = {}
        for gi, (g0, nch) in enumerate(GROUPS):
            for lc in range(nch):
                chunk_map[g0 + lc] = (gi, lc)

        def xs_slice(c, w):
            gi, lc = chunk_map[c]
            return xw_t[gi][:, lc * 128 : lc * 128 + w]

        def ws_slice(c):
            gi, lc = chunk_map[c]
            nch = GROUPS[gi][1]
            off = nch * 128 + lc * 256
            return xw_t[gi][:, off : off + 256]

        def diag_extract(st_ps, tag):
            """[32,64] <- partial diag-block sums of st_ps (engines need
            32-aligned partition bases, so the last 16-offset fold is done
            off-partition: on readback after AllGather, or on the host).
            Result blocks: q=0 at [0:16, 0:32], q=1 at [16:32, 32:64]."""
            # SBUF acc + PSUM operand only (SB+SB needs equal base partitions)
            e1 = small.tile([32, 64], F32, tag="e1")
            nc.vector.tensor_copy(e1[:], st_ps[0:32, 0:64])
            e2 = small.tile([32, 64], F32, tag="e2")
            nc.vector.tensor_add(e2[:], e1[:], st_ps[32:64, 64:128])
            e3 = small.tile([32, 64], F32, tag="e3")
            nc.vector.tensor_add(e3[:], e2[:], st_ps[64:96, 128:192])
            e4 = small.tile([32, 64], F32, tag="e4")
            nc.vector.tensor_add(e4[:], e3[:], st_ps[96:128, 192:256])
            return e4

        def s_sweep(lhs_for_chunk, tag, c0=0, c1=CHUNKS):
            """t[b,(j,o)] = sum_{n,i} y W as accumulated outer products."""
            st_ps = ps_s.tile([128, 256], F32, tag="stacc")
            for c in range(c0, c1):
                nc.tensor.matmul(
                    st_ps[:],
                    lhs_for_chunk(c),
                    ws_slice(c),
                    start=(c == c0),
                    stop=(c == c1 - 1),
                )
            return diag_extract(st_ps, tag)

        epst = small.tile([16, 1], F32, tag="epst")
        nc.vector.memset(epst[:], EPS)

        def squash(s_tile, scale):
            """v = squash(scale * s), s_tile [16,32] viewed [16,2,16]."""
            sq = small.tile([16, 32], F32, tag="sq")
            nc.vector.tensor_mul(sq[:], s_tile[:], s_tile[:])
            sn = small.tile([16, 2], F32, tag="sn")
            nc.vector.tensor_reduce(
                sn[:],
                sq[:].rearrange("p (j o) -> p j o", j=2),
                mybir.AxisListType.X,
                mybir.AluOpType.add,
            )
            sns = small.tile([16, 2], F32, tag="sns")
            nc.vector.tensor_scalar_mul(sns[:], sn[:], scale * scale)
            den = small.tile([16, 2], F32, tag="den")
            nc.vector.tensor_scalar_add(den, sns[:], 1.0)
            rec = small.tile([16, 2], F32, tag="rec")
            nc.vector.reciprocal(rec[:], den[:])
            sr = small.tile([16, 2], F32, tag="sr")
            nc.scalar.activation(sr[:], sns[:], AF.Sqrt, bias=epst[:])
            rs = small.tile([16, 2], F32, tag="rs")
            nc.vector.reciprocal(rs[:], sr[:])
            f = small.tile([16, 2], F32, tag="f")
            nc.vector.tensor_mul(f[:], sns[:], rec[:])
            f2 = small.tile([16, 2], F32, tag="f2")
            nc.vector.tensor_mul(f2[:], f[:], rs[:])
            fs = small.tile([16, 2], F32, tag="fs")
            nc.vector.tensor_scalar_mul(fs[:], f2[:], scale)
            v = small.tile([16, 32], F32, tag=f"v_{scale}_{nc.next_id()}")
            nc.vector.tensor_mul(
                v[:].rearrange("p (j o) -> p j o", j=2),
                s_tile[:].rearrange("p (j o) -> p j o", j=2),
                fs[:].unsqueeze(2).broadcast_to([16, 2, 16]),
            )
            return v

        def ag_send(src_e3, idx):
            """Stage a [32,64] q-blocked partial and trigger its AllGather."""
            a_in, a_out = ag_bufs[idx]
            nc.scalar.dma_start(a_in[:], src_e3[:])
            nc.gpsimd.collective_compute(
                "AllGather",
                mybir.AluOpType.bypass,
                replica_groups=[list(range(num_devices))],
                ins=[a_in[:]],
                outs=[a_out[:]],
            )

        def ag_recv(idx):
            """Read back an AllGather: 2 strided DMAs de-block the q=0/q=1
            summands into [16, (r,q,jo)] = [16,512], then a 4-level
            free-dim tree sums the 16 partials -> [16,32]."""
            a_in, a_out = ag_bufs[idx]
            g = small.tile([16, 512], F32, tag=f"agg{idx}")
            gv = g[:].rearrange("p (r q c) -> p r q c", r=8, q=2)
            av = a_out[:].rearrange("(r p) c -> p r c", p=32)
            for q, eng in ((0, nc.scalar), (1, nc.sync)):
                eng.dma_start(
                    gv[:, :, q, :],
                    av[q * 16 : q * 16 + 16, :, q * 32 : q * 32 + 32],
                )
            h1 = small.tile([16, 256], F32, tag=f"agh1_{idx}")
            nc.vector.tensor_add(h1[:], g[:, 0:256], g[:, 256:512])
            h2 = small.tile([16, 128], F32, tag=f"agh2_{idx}")
            nc.vector.tensor_add(h2[:], h1[:, 0:128], h1[:, 128:256])
            h3 = small.tile([16, 64], F32, tag=f"agh3_{idx}")
            nc.vector.tensor_add(h3[:], h2[:, 0:64], h2[:, 64:128])
            h4 = small.tile([16, 32], F32, tag=f"agh4_{idx}")
            nc.vector.tensor_add(h4[:], h3[:, 0:32], h3[:, 32:64])
            return h4

        def all_gather_sum(src_e3, idx):
            ag_send(src_e3, idx)
            return ag_recv(idx)

        # ---- stage A: st0[b,(j,o)] = sum_{n,i} x W ----
        st0_sb = s_sweep(lambda c: xs_slice(c, 128), "a")
        st0g = all_gather_sum(st0_sb, 0)
        nc.sync.dma_start(s0g_out[:], st0g[:])
        v0 = squash(st0g, 0.5)

        def routing_pass(vacc, it):
            """Given accumulated v [16,32], compute t[b,(j,o)] partial (SBUF)."""
            # Vt transposed + sign: vT[(j,o), b] = +/- vacc[b, (j,o)]
            vt_in = work.tile([32, 32], F32, tag="vt_in")
            nc.vector.memset(vt_in[:], 0.0)
            nc.vector.tensor_copy(vt_in[0:16, 0:16], vacc[:, 0:16])
            nc.vector.tensor_scalar_mul(vt_in[0:16, 16:32], vacc[:, 16:32], -1.0)
            vT = work.tile([32, 32], F32, tag="vT")
            nc.vector.transpose(vT[:], vt_in[:])
            # S4 = I_4 (x) vT : [128 (il,j,o), 64 hi | 64 lo (g,b)]  (fp8)
            # v is split hi+lo across two fp8 planes (z accumulates both)
            # to kill the systematic logit error from quantizing v alone.
            # (pre-zeroed at kernel start; diag blocks rewritten per pass)
            s4 = s4_tiles[it]
            for gg in range(4):
                nc.scalar.copy(
                    s4[gg * 32 : gg * 32 + 32, gg * 16 : gg * 16 + 16],
                    vT[0:32, 0:16],
                )
            vlo = work.tile([32, 16], F32, tag="vlo")
            nc.vector.tensor_sub(vlo[:], vT[0:32, 0:16], s4[0:32, 0:16])
            for gg in range(4):
                nc.scalar.copy(
                    s4[gg * 32 : gg * 32 + 32, 64 + gg * 16 : 64 + gg * 16 + 16],
                    vlo[:],
                )
            # z per 512-n tile (fp8 matmul, hi+lo accumulated), xz (DVE)
            xz_t = []
            for t in range(ZT):
                z_ps = ps_z.tile([128, 512], F32, tag="z")
                for H in (0, 1):
                    for lohi in (0, 1):
                        nc.tensor.matmul(
                            z_ps[H * 64 : H * 64 + 64, :],
                            s4[:, lohi * 64 : lohi * 64 + 64],
                            w4sb[:, t * 1024 + H * 512 : t * 1024 + H * 512 + 512],
                            start=(lohi == 0),
                            stop=(lohi == 1),
                            tile_position=(0, H * 64),
                            skip_group_check=True,
                        )
                xz = xzpool.tile([128, 512], DT, tag="xz")
                nc.vector.tensor_mul(xz[:], z_ps[:], x8sb[:, t * 512 : t * 512 + 512])
                xz_t.append(xz)
            # d (PE delta-matmuls, 4 x [128,16] into one [128,64] bank)
            # + ONE sigmoid ACT per tile
            wtiles = []
            for t in range(ZT):
                d_ps = ps_d.tile([128, 64], F32, tag="d")
                for k4 in range(4):
                    nc.tensor.matmul(
                        d_ps[:, k4 * 16 : k4 * 16 + 16],
                        xz_t[t][:, k4 * 128 : k4 * 128 + 128],
                        oneD[:],
                        start=(k4 == 0),
                        stop=(k4 == 3),
                        skip_group_check=True,
                    )
                w_sb = wpool.tile([128, 64], DT, tag="w")
                nc.scalar.activation(w_sb[:], d_ps[:], AF.Sigmoid)
                wtiles.append(w_sb)

            # y = w * x: one [128, 512] bf16 mul per z-tile (w broadcast over i)
            ytiles = {}

            def y_for_chunk(c):
                t = c // 4
                if t not in ytiles:
                    y4 = ypool.tile([128, 512], DT, tag="y")
                    nc.vector.tensor_mul(
                        y4[:].rearrange("p (k i b) -> p k i b", k=4, i=8),
                        xs_slice(4 * t, 512).rearrange(
                            "p (k i b) -> p k i b", k=4, i=8
                        ),
                        wtiles[t][:]
                        .rearrange("p (k b) -> p k b", k=4)
                        .unsqueeze(2)
                        .broadcast_to([128, 4, 8, 16]),
                    )
                    ytiles[t] = y4
                return ytiles[t][:, (c % 4) * 128 : (c % 4) * 128 + 128]

            if it == 1:
                # split sweep: AG of the first half overlaps the second half
                eA = s_sweep(y_for_chunk, "i1a", 0, 48)
                ag_send(eA, 1)
                eB = s_sweep(y_for_chunk, "i1b", 48, CHUNKS)
                ag_send(eB, 2)
                return None
            return s_sweep(y_for_chunk, f"i{it}")

        # ---- iteration 1 ----
        routing_pass(v0, 1)
        hA = ag_recv(1)
        hB = ag_recv(2)
        t1g = small.tile([16, 32], F32, tag="t1g")
        nc.vector.tensor_add(t1g[:], hA[:], hB[:])
        s1 = small.tile([16, 32], F32, tag="s1")
        nc.vector.tensor_copy(s1[:, 0:16], t1g[:, 0:16])
        nc.vector.tensor_sub(s1[:, 16:32], st0g[:, 16:32], t1g[:, 16:32])
        v1 = squash(s1, 1.0)
        vacc2 = small.tile([16, 32], F32, tag="vacc2")
        nc.vector.tensor_add(vacc2[:], v0[:], v1[:])

        # ---- iteration 2 (partials out; host combines) ----
        t2_sb = routing_pass(vacc2, 2)
        nc.sync.dma_start(t2_out[:], t2_sb[:])

    nc.compile()
    return nc


def _squash_np(s):
    sn = np.sum(s * s, axis=-1, keepdims=True)
    return sn / (1.0 + sn) / np.sqrt(sn + EPS) * s


def finish_host(results):
    """Combine per-core (t2 [32,64] q-blocked, s0g) partials into v2."""
    t2b = sum(np.asarray(r["t2"], dtype=np.float64) for r in results)
    t2 = t2b[0:16, 0:32] + t2b[16:32, 32:64]
    s0g = np.asarray(results[0]["s0g"], dtype=np.float64)
    s2 = np.empty((16, 2, 16), dtype=np.float64)
    s2[:, 0, :] = t2[:, 0:16]
    s2[:, 1, :] = s0g[:, 16:32] - t2[:, 16:32]
    return _squash_np(s2).astype(np.float32)


def run(x, W, **spmd_kwargs):
    from concourse.bass_utils import run_bass_kernel_spmd

    x = np.asarray(x, dtype=np.float32)
    W = np.asarray(W, dtype=np.float32)
    in_maps = host_prep(x, W)
    key = "nc_v2"
    if key not in _CACHE:
        _CACHE[key] = build_kernel()
    nc = _CACHE[key]
    res = run_bass_kernel_spmd(nc, in_maps, list(range(N_CORES)), **spmd_kwargs)
    return finish_host(res.results), res


def kernel(x, W):
    return run(x, W)[0]
